# revision 1
# baseline (speedup 1.0000x reference)
"""Multi-head attention Trainium2 Bass kernel.

Shapes (hardcoded): B=4, T=2048, E=1024, H=16, DK=64.
Sharding over 8 cores: core c -> (batch b = c//2, head-group g = c%2).
Each core computes 8 heads of one batch end-to-end and a partial output
projection; the host sums the two partials per batch.

Layout strategy (everything transposed so no on-device transposes):
  - inputs fed as x^T [E, T] (host-transposed)
  - Q^T, K^T kept as [f_local, T] (f on partitions)
  - V kept natural [T, f_local], stored per-head with an appended
    ones-column so attn@V also produces softmax row-sums in PSUM row 64
  - S^T [keys, q] per (head, key-tile); exp fused with 1/sqrt(dk) scale and
    additive mask bias (per-partition) on the scalar engine
  - output projection consumes x^T_local directly as lhsT
"""

import numpy as np

import concourse.bass as bass
import concourse.tile as tile
from concourse import bacc, mybir
from concourse.bass_utils import run_bass_kernel_spmd

F32 = mybir.dt.float32
F32R = mybir.dt.float32r

B, T, E, H = 4, 2048, 1024, 16
DK = E // H            # 64
N_CORES = 8
FL = 512               # local f (8 heads * 64)
HL = 8                 # heads per core
NT = T // 128          # 16 t-tiles
NE = E // 128          # 8 e-tiles
NFT = FL // 128        # 4 local f-tiles
NC4 = T // 512         # 4 t-chunks of 512

BF16 = mybir.dt.bfloat16
DT = BF16


def build_nc():
    nc = bacc.Bacc("TRN2", target_bir_lowering=False, debug=False,
                   enable_asserts=False)

    qT = nc.dram_tensor("qT", [E, T], DT, kind="ExternalInput").ap()
    kT = nc.dram_tensor("kT", [E, T], DT, kind="ExternalInput").ap()
    vT = nc.dram_tensor("vT", [E, T], DT, kind="ExternalInput").ap()
    wqT = nc.dram_tensor("wqT", [E, FL], DT, kind="ExternalInput").ap()
    wkT = nc.dram_tensor("wkT", [E, FL], DT, kind="ExternalInput").ap()
    wvT = nc.dram_tensor("wvT", [E, FL], DT, kind="ExternalInput").ap()
    woT = nc.dram_tensor("woT", [FL, E], DT, kind="ExternalInput").ap()
    bq = nc.dram_tensor("bq", [128, NFT], F32, kind="ExternalInput").ap()
    bk = nc.dram_tensor("bk", [128, NFT], F32, kind="ExternalInput").ap()
    bv = nc.dram_tensor("bv", [1, FL], DT, kind="ExternalInput").ap()
    bo = nc.dram_tensor("bo", [1, E], DT, kind="ExternalInput").ap()
    ones_d = nc.dram_tensor("ones_d", [1, 128], DT, kind="ExternalInput").ap()
    # per-key 0/1 mask, replicated per head: vmask[p, j*HL+h] = mask[j*128+p]
    vmask = nc.dram_tensor("vmask", [128, NT * HL], DT,
                           kind="ExternalInput").ap()
    vmaskf = nc.dram_tensor("vmaskf", [128, NT], F32,
                            kind="ExternalInput").ap()
    out = nc.dram_tensor("out", [T, E], F32, kind="ExternalOutput").ap()

    with tile.TileContext(nc) as tc:
        with (
            tc.tile_pool(name="const", bufs=1) as constp,
            tc.tile_pool(name="qkt", bufs=1) as qktp,
            tc.tile_pool(name="vsb", bufs=1) as vsbp,
            tc.tile_pool(name="xtl", bufs=1) as xtlp,
            tc.tile_pool(name="ps_s", bufs=2, space="PSUM") as ps_s,
            tc.tile_pool(name="ps_o", bufs=1, space="PSUM") as ps_o,
            tc.tile_pool(name="ps_w", bufs=1, space="PSUM") as ps_w,
        ):
            # ---- constants ----
            bq_sb = constp.tile([128, NFT], F32, tag="bq")
            nc.sync.dma_start(out=bq_sb[:], in_=bq)
            bk_sb = constp.tile([128, NFT], F32, tag="bk")
            nc.sync.dma_start(out=bk_sb[:], in_=bk)
            bv_sb = constp.tile([1, FL], DT, tag="bv")
            nc.sync.dma_start(out=bv_sb[:], in_=bv)
            bo_sb = constp.tile([1, E], DT, tag="bo")
            nc.sync.dma_start(out=bo_sb[:], in_=bo)
            ones_sb = constp.tile([1, 128], DT, tag="ones")
            nc.sync.dma_start(out=ones_sb[:], in_=ones_d)
            vmask_sb = constp.tile([128, NT * HL], DT, tag="vmask")
            nc.sync.dma_start(out=vmask_sb[:], in_=vmask)
            vmaskf_sb = constp.tile([128, NT], F32, tag="vmaskf")
            nc.sync.dma_start(out=vmaskf_sb[:], in_=vmaskf)

            # persistent activation storage: per-head [128, T] tiles with
            # rows 64..127 = copy of rows 0..63 (dk), so the S matmuls can
            # run as row-tiled pairs: PE rows 0-63 compute k-tile 2i while
            # rows 64-127 compute k-tile 2i+1 in the same 512-cycle stream
            qt = [qktp.tile([128, T], DT, tag=f"qt{i}", name=f"qt{i}")
                  for i in range(HL)]
            kt = [qktp.tile([128, T], DT, tag=f"kt{i}", name=f"kt{i}")
                  for i in range(HL)]
            # V per t-tile: [128, 8 heads * 65]; per head: cols 0..63 = V,
            # col 64 = mask01 (row-sum trick; masked keys contribute 0)
            vt = [vsbp.tile([128, HL * 65], DT, tag=f"v{j}", name=f"v{j}")
                  for j in range(NT)]
            xtl = [xtlp.tile([128, T], DT, tag=f"x{i}", name=f"x{i}")
                   for i in range(NFT)]

            # ---- pools ----
            wp = tc.alloc_tile_pool(name="wqk", bufs=1)
            xlp = tc.alloc_tile_pool(name="xload", bufs=9)
            expp = tc.alloc_tile_pool(name="exps", bufs=6)
            normp = tc.alloc_tile_pool(name="norm", bufs=2)
            normdp = tc.alloc_tile_pool(name="normd", bufs=2, space="DRAM")
            wvp = tc.alloc_tile_pool(name="wv", bufs=1)
            vlp = tc.alloc_tile_pool(name="vload", bufs=1)

            w_sb = {}
            for name, wdram in (("k", wkT), ("q", wqT)):
                w_sb[name] = [
                    wp.tile([128, FL], DT, tag=f"w{name}{e}",
                            name=f"w{name}{e}") for e in range(NE)]
                for e in range(NE):
                    nc.sync.dma_start(
                        out=w_sb[name][e][:],
                        in_=wdram[e * 128:(e + 1) * 128, :])
            wv_sb = [wvp.tile([128, FL], DT, tag=f"wv{e}", name=f"wv{e}")
                     for e in range(NE)]
            for e in range(NE):
                nc.sync.dma_start(out=wv_sb[e][:],
                                  in_=wvT[e * 128:(e + 1) * 128, :])

            # V projection half (natural layout + mask col); psum comes
            # from the ps_w pool so it cannot deadlock against the
            # attention pairs' ps_s rotation
            def v_proj(hf):
                vf = [vlp.tile([128, 1024], DT, tag=f"vf{e}",
                               name=f"vf{e}") for e in range(NE)]
                for e in range(NE):
                    nc.sync.dma_start(
                        out=vf[e][:],
                        in_=vT[e * 128:(e + 1) * 128,
                               hf * 1024:(hf + 1) * 1024])
                for jj in range(NT // 2):
                    j = hf * (NT // 2) + jj
                    ps = ps_w.tile([128, 1024], F32, tag="psqk",
                                   name="psv")
                    for e in range(NE):
                        nc.tensor.matmul(
                            ps[:, 0:FL],
                            lhsT=vf[e][:, jj * 128:(jj + 1) * 128],
                            rhs=wv_sb[e][:],
                            start=(e == 0), stop=False)
                    nc.tensor.matmul(ps[:, 0:FL], lhsT=ones_sb[:],
                                     rhs=bv_sb[:], start=False, stop=True)
                    nc.sync.dma_start(
                        out=vt[j].rearrange(
                            "p (h w) -> p h w", w=65)[:, :, 64:65],
                        in_=vmask_sb[:, j * HL:(j + 1) * HL].rearrange(
                            "p (h o) -> p h o", o=1))
                    nc.vector.tensor_scalar_mul(
                        vt[j].rearrange(
                            "p (h w) -> p h w", w=65)[:, :, 0:64],
                        ps[:, 0:FL].rearrange(
                            "p (h w) -> p h w", w=64),
                        vmaskf_sb[:, j:j + 1])

            def proj_pass(fl, name, demote=0):
                xdram = kT if name == "k" else qT
                bias_sb = bk_sb if name == "k" else bq_sb
                dst = kt if name == "k" else qt
                for c in range(NC4):
                    xs = []
                    for e in range(NE):
                        xe = xlp.tile([128, 512], DT, tag="xchunk",
                                      name="xchunk")
                        nc.sync.dma_start(
                            out=xe[:],
                            in_=xdram[e * 128:(e + 1) * 128,
                                      c * 512:(c + 1) * 512])
                        xs.append(xe)
                    save = tc.cur_priority
                    if demote:
                        tc.cur_priority = save + demote
                    ps = ps_w.tile([128, 512 * len(fl)], F32, tag="psqk",
                                   name="psqk")
                    for fi, f in enumerate(fl):
                        for e in range(NE):
                            nc.tensor.matmul(
                                ps[:, fi * 512:(fi + 1) * 512],
                                lhsT=w_sb[name][e][:, f * 128:(f + 1) * 128],
                                rhs=xs[e][:],
                                start=(e == 0), stop=(e == NE - 1))
                    for fi, f in enumerate(fl):
                        for hh in range(2):
                            nc.vector.tensor_scalar_add(
                                dst[2 * f + hh][0:64,
                                                c * 512:(c + 1) * 512],
                                ps[hh * 64:(hh + 1) * 64,
                                   fi * 512:(fi + 1) * 512],
                                bias_sb[hh * 64:(hh + 1) * 64,
                                        f:f + 1])
                    if demote:
                        tc.cur_priority = save
                # duplicate dk rows into partitions 64..127 for row-tiled S
                for f in fl:
                    for hh in range(2):
                        nc.sync.dma_start(
                            out=dst[2 * f + hh][64:128, :],
                            in_=dst[2 * f + hh][0:64, :])

            def attention_half(h, half):
                qh = qt[h]
                kh = kt[h]
                if True:
                    qsl = slice(half * 1024, (half + 1) * 1024)
                    pso = ps_o.tile([128, 1024], F32, tag="ps_o",
                                    name="pso")
                    for kp in range(NT // 2):
                        for j in range(2):
                            # row-tiled S^T pair: PE rows 0-63 compute
                            # k-tile 2kp, rows 64-127 compute k-tile
                            # 2kp+1, streaming the same q-chunk
                            # concurrently. One [128, 1024] slot per pair
                            # so both members become ready together
                            # (separate slots would stagger them by one
                            # exp duration).
                            cj = half * 1024 + j * 512
                            pss = ps_s.tile([128, 1024], F32, tag="ps_s",
                                            name="pss")
                            for t in range(2):
                                k = 2 * kp + t
                                r = slice(t * 64, t * 64 + 64)
                                nc.tensor.matmul(
                                    pss[:, t * 512:(t + 1) * 512],
                                    lhsT=kh[r, k * 128:(k + 1) * 128],
                                    rhs=qh[r, cj:cj + 512],
                                    start=True, stop=True,
                                    tile_position=(t * 64, 0))
                            es = expp.tile([128, 1024], DT, tag="es",
                                           name="es")
                            nc.scalar.activation(
                                out=es[:], in_=pss[:],
                                func=mybir.ActivationFunctionType.Exp,
                                scale=0.125)
                            for t in range(2):
                                k = 2 * kp + t
                                nc.tensor.matmul(
                                    pso[0:65, j * 512:(j + 1) * 512],
                                    lhsT=vt[k][:, h * 65:(h + 1) * 65],
                                    rhs=es[:, t * 512:(t + 1) * 512],
                                    start=(kp == 0 and t == 0),
                                    stop=(kp == NT // 2 - 1 and t == 1))
                    # normalize: rows 0..63 = O^T, row 64 = sum(exp).
                    # scatter the [1, 1024] sums row into [64, 16] via a
                    # DRAM bounce so the DVE reciprocal runs on 64 lanes
                    # (a [1, 1024] reciprocal is serial: ~6.5us)
                    ot = normp.tile([65, 1024], F32, tag="ot", name="ot")
                    nc.vector.tensor_copy(out=ot[:], in_=pso[0:65, :])
                    rsd = normdp.tile([1, 1024], F32, tag="rsd",
                                      name="rsd")
                    nc.sync.dma_start(out=rsd[:], in_=ot[64:65, :])
                    rs = normp.tile([64, 16], F32, tag="rs", name="rs")
                    nc.sync.dma_start(
                        out=rs[:],
                        in_=rsd.rearrange("o (p w) -> (o p) w", w=16))
                    ri = normp.tile([64, 16], F32, tag="ri", name="ri")
                    nc.vector.reciprocal(ri[:], rs[:])
                    rid = normdp.tile([64, 16], F32, tag="rid",
                                      name="rid")
                    nc.sync.dma_start(out=rid[:], in_=ri[:])
                    rep = normp.tile([64, 1024], F32, tag="rep",
                                     name="rep")
                    nc.sync.dma_start(
                        out=rep[0:1, :],
                        in_=rid.rearrange("p w -> () (p w)"))
                    for d in range(6):  # 1 -> 64 partitions
                        w = 1 << d
                        nc.sync.dma_start(out=rep[w:2 * w, :],
                                          in_=rep[0:w, :])
                    nc.vector.tensor_mul(
                        xtl[h // 2][h % 2 * 64:h % 2 * 64 + 64, qsl],
                        ot[0:64, :], rep[:])

            proj_pass([0], "k")
            proj_pass([0], "q")

            # ---- phase 3: output projection (partial) ----
            wop = tc.alloc_tile_pool(name="wo", bufs=1)
            osbp = tc.alloc_tile_pool(name="osb", bufs=2)
            wo_sb = [wop.tile([128, E], DT, tag=f"wo{e}", name=f"wo{e}")
                     for e in range(NFT)]
            for e in range(NFT):
                nc.sync.dma_start(out=wo_sb[e][:],
                                  in_=woT[e * 128:(e + 1) * 128, :])

            def final_proj(js, demote=0):
                save = tc.cur_priority
                if demote:
                    tc.cur_priority = save + demote
                for j in js:
                    ps = ps_w.tile([128, E], F32, tag="psqk", name="psf")
                    for c2 in range(2):
                        nc.tensor.matmul(
                            ps[:, c2 * 512:(c2 + 1) * 512],
                            lhsT=ones_sb[:],
                            rhs=bo_sb[:, c2 * 512:(c2 + 1) * 512],
                            start=True, stop=False)
                    for e in range(NFT):
                        for c2 in range(2):
                            nc.tensor.matmul(
                                ps[:, c2 * 512:(c2 + 1) * 512],
                                lhsT=xtl[e][:, j * 128:(j + 1) * 128],
                                rhs=wo_sb[e][:, c2 * 512:(c2 + 1) * 512],
                                start=False, stop=(e == NFT - 1))
                    ob = osbp.tile([128, E], F32, tag="ob", name="ob")
                    nc.vector.tensor_copy(out=ob[:], in_=ps[:])
                    nc.sync.dma_start(out=out[j * 128:(j + 1) * 128, :],
                                      in_=ob[:])
                tc.cur_priority = save

            # phase A: all heads, first q-half; then final rows 0..1023.
            # (order here IS the dataflow: a read sees the most recent
            # write before it in program order, so attention must follow
            # the projections that feed it). Each K/Q pass is emitted one
            # attention half ahead of its consumer with its matmuls
            # priority-demoted, so the PE prefers the S-pairs that feed
            # the ACT exp stream and the projections soak up PE slack.
            v_proj(0)
            v_proj(1)
            attention_half(0, 0)
            proj_pass([1], "k", demote=60)
            proj_pass([1], "q", demote=120)
            attention_half(1, 0)
            attention_half(2, 0)
            proj_pass([2], "k", demote=60)
            proj_pass([2], "q", demote=120)
            attention_half(3, 0)
            attention_half(4, 0)
            proj_pass([3], "k", demote=60)
            proj_pass([3], "q", demote=120)
            attention_half(5, 0)
            attention_half(6, 0)
            attention_half(7, 0)
            final_proj(range(NT // 2), demote=250)
            # phase B: second q-half; then final rows 1024..2047
            for h in range(HL):
                attention_half(h, 1)
            final_proj(range(NT // 2, NT))
            for p in (osbp, wop, vlp, wvp, normdp, normp, expp, xlp, wp):
                p.release()

    nc.compile()
    return nc


_NC_CACHE = None


def _get_nc():
    global _NC_CACHE
    if _NC_CACHE is None:
        _NC_CACHE = build_nc()
    return _NC_CACHE


def make_in_maps(query, key_, value, mask, w_q, b_q, w_k, b_k, w_v, b_v,
                 w_o, b_o):
    import ml_dtypes
    f32 = np.float32
    bf16 = ml_dtypes.bfloat16
    c = lambda a: np.ascontiguousarray(a).astype(bf16)
    in_maps = []
    for core in range(N_CORES):
        b, g = core // 2, core % 2
        fs = slice(g * FL, (g + 1) * FL)
        # per-key 0/1 mask -> [128, NT*HL]: vmask[p, j*HL+h] = mask[j*128+p]
        m01 = mask[b].astype(f32).reshape(NT, 128).T  # [128, NT]
        vm = np.repeat(m01[:, :, None], HL, axis=2).reshape(128, NT * HL)
        in_maps.append({
            "qT": c(query[b].T.astype(f32, copy=False)),
            "kT": c(key_[b].T.astype(f32, copy=False)),
            "vT": c(value[b].T.astype(f32, copy=False)),
            "wqT": c(w_q[fs, :].T.astype(f32, copy=False)),
            "wkT": c(w_k[fs, :].T.astype(f32, copy=False)),
            "wvT": c(w_v[fs, :].T.astype(f32, copy=False)),
            "woT": c(w_o[:, fs].T.astype(f32, copy=False)),
            "bq": np.ascontiguousarray(
                b_q[fs].astype(f32, copy=False).reshape(NFT, 128).T),
            "bk": np.ascontiguousarray(
                b_k[fs].astype(f32, copy=False).reshape(NFT, 128).T),
            "bv": b_v[fs].reshape(1, FL).astype(bf16),
            "bo": (b_o.astype(f32, copy=False) if g == 0
                   else np.zeros(E, f32)).reshape(1, E).astype(bf16),
            "ones_d": np.ones((1, 128), bf16),
            "vmask": vm.astype(bf16),
            "vmaskf": np.ascontiguousarray(m01),
        })
    return in_maps


def kernel(query=None, key_=None, value=None, mask=None, w_q=None, b_q=None,
           w_k=None, b_k=None, w_v=None, b_v=None, w_o=None, b_o=None,
           key=None, **_kwargs):
    if key_ is None:
        key_ = key
    args = [np.asarray(a) for a in
            (query, key_, value, mask, w_q, b_q, w_k, b_k, w_v, b_v,
             w_o, b_o)]
    nc = _get_nc()
    in_maps = make_in_maps(*args)
    res = run_bass_kernel_spmd(nc, in_maps, core_ids=list(range(N_CORES)))
    outs = [res.results[i]["out"] for i in range(N_CORES)]
    full = np.empty((B, T, E), np.float32)
    for b in range(B):
        full[b] = outs[2 * b] + outs[2 * b + 1]
    return full



# revision 4
# speedup vs baseline: 1.1185x; 1.1185x over previous
"""Multi-head attention Trainium2 Bass kernel (v2).

Shapes (hardcoded): B=4, T=2048, E=1024, H=16, DK=64.
Sharding over 8 cores: core c -> (batch b = c//2, head-group g = c%2).
Each core computes 8 heads of one batch end-to-end and a partial output
projection; the host sums the two partials per batch.

v2 design (ACT-saturation oriented; the exp stream is the roofline):
  - head-PAIR S matmuls: K/Q stored as natural f-tiles [128, T] where
    rows 0:64 = head 2i's dk and rows 64:128 = head 2i+1's dk. The S
    matmul for a key tile is a row-tiled PE pair (tile_position (0,0) /
    (64,0)) computing BOTH heads concurrently in one 512-cycle stream.
    No row duplication DMAs needed.
  - key mask applied as a per-partition bias operand of the exp
    activation (keys live on partitions of S^T): masked keys get
    bias=-30 => exp ~ 0, so they drop out of both attn@V and the
    row-sum column. V needs no masking.
  - every x chunk is loaded exactly once (f-loop inside chunk loop).
  - exp input tiles are [128, 1024] PSUM (2 banks), double buffered;
    attn@V accumulates per-head [65, 512] PSUM tiles (V plus a ones
    column producing the softmax row sums in row 64).
  - softmax normalization: row-sums bounce through DRAM to turn the
    [1, 512] sums row into [64, 16] lanes for the DVE reciprocal, then
    a stride-0 broadcast DMA replicates the reciprocals to 64
    partitions for the normalize multiply.
  - output projection per q-chunk with the bias added by the DVE
    (tensor_add with a pre-replicated bias tile) during the PSUM->SBUF
    move; no bias matmuls.
"""

import numpy as np

import concourse.bass as bass
import concourse.tile as tile
from concourse import bacc, mybir
from concourse.bass_utils import run_bass_kernel_spmd

F32 = mybir.dt.float32
BF16 = mybir.dt.bfloat16
DT = BF16

B, T, E, H = 4, 2048, 1024, 16
DK = E // H            # 64
N_CORES = 8
FL = 512               # local f (8 heads * 64)
HL = 8                 # heads per core
HP = HL // 2           # head pairs per core = f tiles
NT = T // 128          # 16 key tiles
NE = E // 128          # 8 e tiles
NC = T // 512          # 4 chunks of 512

# PE priority classes (lower = preferred by the scheduler)
PRI_ATTNV = 40
PRI_PROJ = 80
PRI_FINAL = 200


def build_nc():
    nc = bacc.Bacc("TRN2", target_bir_lowering=False, debug=False,
                   enable_asserts=False)

    qT = nc.dram_tensor("qT", [E, T], DT, kind="ExternalInput").ap()
    kT = nc.dram_tensor("kT", [E, T], DT, kind="ExternalInput").ap()
    vT = nc.dram_tensor("vT", [E, T], DT, kind="ExternalInput").ap()
    wqT = nc.dram_tensor("wqT", [E, FL], DT, kind="ExternalInput").ap()
    wkT = nc.dram_tensor("wkT", [E, FL], DT, kind="ExternalInput").ap()
    wvT = nc.dram_tensor("wvT", [E, FL], DT, kind="ExternalInput").ap()
    woT = nc.dram_tensor("woT", [FL, E], DT, kind="ExternalInput").ap()
    bqc = nc.dram_tensor("bqc", [128, HP], F32, kind="ExternalInput").ap()
    bkc = nc.dram_tensor("bkc", [128, HP], F32, kind="ExternalInput").ap()
    bvr = nc.dram_tensor("bvr", [128, FL], DT, kind="ExternalInput").ap()
    bor = nc.dram_tensor("bor", [128, E], DT, kind="ExternalInput").ap()
    # per-key additive exp bias: 0 (allowed) or -30 (masked)
    mbd = nc.dram_tensor("mbd", [128, NT], F32, kind="ExternalInput").ap()
    out = nc.dram_tensor("out", [T, E], F32, kind="ExternalOutput").ap()

    with tile.TileContext(nc) as tc:
        with (
            tc.tile_pool(name="const", bufs=1) as constp,
            tc.tile_pool(name="qkt", bufs=1) as qktp,
            tc.tile_pool(name="vsb", bufs=1) as vsbp,
            tc.tile_pool(name="xtl", bufs=1) as xtlp,
            tc.tile_pool(name="wgt", bufs=1) as wp,
            tc.tile_pool(name="xk", bufs=2) as xkp,
            tc.tile_pool(name="xq", bufs=2) as xqp,
            tc.tile_pool(name="xv", bufs=2) as xvp,
            tc.tile_pool(name="es", bufs=8) as esp,
            tc.tile_pool(name="norm", bufs=2) as normp,
            tc.tile_pool(name="normd", bufs=2, space="DRAM") as normdp,
            tc.tile_pool(name="ob", bufs=4) as obp,
            tc.tile_pool(name="ps_s", bufs=2, space="PSUM") as ps_s,
            tc.tile_pool(name="ps_o", bufs=1, space="PSUM") as ps_o,
            tc.tile_pool(name="ps_w", bufs=2, space="PSUM") as ps_w,
        ):
            # ---- constants ----
            bq_sb = constp.tile([128, HP], F32, tag="bq")
            nc.sync.dma_start(out=bq_sb[:], in_=bqc)
            bk_sb = constp.tile([128, HP], F32, tag="bk")
            nc.sync.dma_start(out=bk_sb[:], in_=bkc)
            bv_sb = constp.tile([128, FL], DT, tag="bv")
            nc.sync.dma_start(out=bv_sb[:], in_=bvr)
            bo_sb = constp.tile([128, E], DT, tag="bo")
            nc.sync.dma_start(out=bo_sb[:], in_=bor)
            mb_sb = constp.tile([128, NT], F32, tag="mb")
            nc.sync.dma_start(out=mb_sb[:], in_=mbd)

            # persistent activations
            kt = [qktp.tile([128, T], DT, tag=f"kt{f}", name=f"kt{f}")
                  for f in range(HP)]
            qt = [qktp.tile([128, T], DT, tag=f"qt{f}", name=f"qt{f}")
                  for f in range(HP)]
            # V per key tile: [128 keys, 8 heads * 65]; per head cols
            # 0..63 = V, col 64 = ones (softmax row-sum trick)
            vt = [vsbp.tile([128, HL * 65], DT, tag=f"v{j}", name=f"v{j}")
                  for j in range(NT)]
            xtl = [xtlp.tile([128, T], DT, tag=f"x{f}", name=f"x{f}")
                   for f in range(HP)]
            for j in range(NT):
                nc.vector.memset(
                    vt[j].rearrange("p (h w) -> p h w", w=65)[:, :, 64:65],
                    1.0)

            # ---- weights ----
            wk_sb = [wp.tile([128, FL], DT, tag=f"wk{e}", name=f"wk{e}")
                     for e in range(NE)]
            wq_sb = [wp.tile([128, FL], DT, tag=f"wq{e}", name=f"wq{e}")
                     for e in range(NE)]
            wv_sb = [wp.tile([128, FL], DT, tag=f"wv{e}", name=f"wv{e}")
                     for e in range(NE)]
            wo_sb = [wp.tile([128, E], DT, tag=f"wo{f}", name=f"wo{f}")
                     for f in range(HP)]
            for e in range(NE):
                nc.sync.dma_start(out=wk_sb[e][:],
                                  in_=wkT[e * 128:(e + 1) * 128, :])
            for e in range(NE):
                nc.sync.dma_start(out=wq_sb[e][:],
                                  in_=wqT[e * 128:(e + 1) * 128, :])
            for e in range(NE):
                nc.sync.dma_start(out=wv_sb[e][:],
                                  in_=wvT[e * 128:(e + 1) * 128, :])
            for f in range(HP):
                nc.sync.dma_start(out=wo_sb[f][:],
                                  in_=woT[f * 128:(f + 1) * 128, :])

            def demote(off):
                save = tc.cur_priority
                tc.cur_priority = save + off
                return save

            def qk_proj(c, xdram, w_sb, bias_sb, dst, pri):
                save = demote(pri)
                xs = []
                for e in range(NE):
                    xe = (xkp if dst is kt else xqp).tile(
                        [128, 512], DT, tag=f"x{e}", name=f"x{e}")
                    nc.sync.dma_start(
                        out=xe[:],
                        in_=xdram[e * 128:(e + 1) * 128,
                                  c * 512:(c + 1) * 512])
                    xs.append(xe)
                for f in range(HP):
                    ps = ps_w.tile([128, 512], F32, tag="psw", name="psw")
                    for e in range(NE):
                        nc.tensor.matmul(
                            ps[:],
                            lhsT=w_sb[e][:, f * 128:(f + 1) * 128],
                            rhs=xs[e][:],
                            start=(e == 0), stop=(e == NE - 1))
                    nc.vector.tensor_scalar_add(
                        dst[f][:, c * 512:(c + 1) * 512],
                        ps[:], bias_sb[:, f:f + 1])
                tc.cur_priority = save

            def v_proj(c, pri):
                save = demote(pri)
                xs = []
                for e in range(NE):
                    xe = xvp.tile([128, 512], DT, tag=f"x{e}", name=f"x{e}")
                    nc.sync.dma_start(
                        out=xe[:],
                        in_=vT[e * 128:(e + 1) * 128,
                               c * 512:(c + 1) * 512])
                    xs.append(xe)
                for jj in range(4):
                    j = 4 * c + jj
                    ps = ps_w.tile([128, 512], F32, tag="psw", name="psw")
                    for e in range(NE):
                        nc.tensor.matmul(
                            ps[:],
                            lhsT=xs[e][:, jj * 128:(jj + 1) * 128],
                            rhs=wv_sb[e][:],
                            start=(e == 0), stop=(e == NE - 1))
                    nc.vector.tensor_add(
                        vt[j].rearrange("p (h w) -> p h w", w=65)[:, :, 0:64],
                        ps.rearrange("p (h w) -> p h w", w=64),
                        bv_sb.rearrange("p (h w) -> p h w", w=64))
                tc.cur_priority = save

            # ---- projections (all emitted up front, need-ordered) ----
            qk_proj(0, kT, wk_sb, bk_sb, kt, PRI_PROJ)
            qk_proj(0, qT, wq_sb, bq_sb, qt, PRI_PROJ)
            qk_proj(1, kT, wk_sb, bk_sb, kt, PRI_PROJ)
            v_proj(0, PRI_PROJ)
            qk_proj(2, kT, wk_sb, bk_sb, kt, PRI_PROJ)
            v_proj(1, PRI_PROJ)
            qk_proj(3, kT, wk_sb, bk_sb, kt, PRI_PROJ)
            v_proj(2, PRI_PROJ)
            v_proj(3, PRI_PROJ)
            qk_proj(1, qT, wq_sb, bq_sb, qt, PRI_PROJ)
            qk_proj(2, qT, wq_sb, bq_sb, qt, PRI_PROJ)
            qk_proj(3, qT, wq_sb, bq_sb, qt, PRI_PROJ)

            # ---- attention + output projection ----
            def unit(hp, qc):
                """One (head-pair, q-chunk-512) attention unit."""
                qsl = slice(qc * 512, (qc + 1) * 512)
                psoA = ps_o.tile([65, 512], F32, tag="psoA", name="psoA")
                psoB = ps_o.tile([65, 512], F32, tag="psoB", name="psoB")
                for k in range(NT):
                    pss = ps_s.tile([128, 1024], F32, tag="pss",
                                    name="pss")
                    for t in range(2):
                        r = slice(t * 64, (t + 1) * 64)
                        nc.tensor.matmul(
                            pss[:, t * 512:(t + 1) * 512],
                            lhsT=kt[hp][r, k * 128:(k + 1) * 128],
                            rhs=qt[hp][r, qsl],
                            start=True, stop=True,
                            tile_position=(t * 64, 0))
                    es = esp.tile([128, 1024], DT, tag="es", name="es")
                    nc.scalar.activation(
                        out=es[:], in_=pss[:],
                        func=mybir.ActivationFunctionType.Exp,
                        bias=mb_sb[:, k:k + 1], scale=0.125)
                    save = demote(PRI_ATTNV)
                    h0 = 2 * hp
                    nc.tensor.matmul(
                        psoA[:], lhsT=vt[k][:, h0 * 65:(h0 + 1) * 65],
                        rhs=es[:, 0:512],
                        start=(k == 0), stop=(k == NT - 1))
                    nc.tensor.matmul(
                        psoB[:], lhsT=vt[k][:, (h0 + 1) * 65:(h0 + 2) * 65],
                        rhs=es[:, 512:1024],
                        start=(k == 0), stop=(k == NT - 1))
                    tc.cur_priority = save
                # normalize: rows 0..63 = O^T, row 64 = sum(exp).
                # DMA can't read PSUM, so DVE-copy the sum rows to SBUF
                # (partition 64 stays partition 64), then scatter to 64
                # partitions for a parallel reciprocal.
                sm = normp.tile([65, 1024], F32, tag="sm", name="sm")
                nc.vector.tensor_copy(out=sm[64:65, 0:512],
                                      in_=psoA[64:65, :])
                nc.vector.tensor_copy(out=sm[64:65, 512:1024],
                                      in_=psoB[64:65, :])
                rsd = normdp.tile([1, 1024], F32, tag="rsd", name="rsd")
                nc.sync.dma_start(out=rsd[:], in_=sm[64:65, :])
                rs = normp.tile([64, 16], F32, tag="rs", name="rs")
                nc.sync.dma_start(
                    out=rs[:],
                    in_=rsd.rearrange("o (p w) -> (o p) w", w=16))
                ri = normp.tile([64, 16], F32, tag="ri", name="ri")
                nc.vector.reciprocal(ri[:], rs[:])
                rid = normdp.tile([64, 16], F32, tag="rid", name="rid")
                nc.sync.dma_start(out=rid[:], in_=ri[:])
                rif = rid.rearrange("p w -> () (p w)")
                repA = normp.tile([64, 512], F32, tag="repA", name="repA")
                nc.sync.dma_start(out=repA[:],
                                  in_=rif[:, 0:512].to_broadcast([64, 512]))
                repB = normp.tile([64, 512], F32, tag="repB", name="repB")
                nc.sync.dma_start(out=repB[:],
                                  in_=rif[:, 512:1024].to_broadcast([64, 512]))
                nc.vector.tensor_mul(
                    xtl[hp][0:64, qsl], psoA[0:64, :], repA[:])
                nc.vector.tensor_mul(
                    xtl[hp][64:128, qsl], psoB[0:64, :], repB[:])

            def final_proj(qc):
                save = demote(PRI_FINAL)
                for j in range(4 * qc, 4 * qc + 4):
                    for c2 in range(2):
                        ps = ps_w.tile([128, 512], F32, tag="psw",
                                       name="psf")
                        for f in range(HP):
                            nc.tensor.matmul(
                                ps[:],
                                lhsT=xtl[f][:, j * 128:(j + 1) * 128],
                                rhs=wo_sb[f][:, c2 * 512:(c2 + 1) * 512],
                                start=(f == 0), stop=(f == HP - 1))
                        ob = obp.tile([128, 512], F32, tag="ob", name="ob")
                        nc.vector.tensor_add(
                            ob[:], ps[:],
                            bo_sb[:, c2 * 512:(c2 + 1) * 512])
                        nc.sync.dma_start(
                            out=out[j * 128:(j + 1) * 128,
                                    c2 * 512:(c2 + 1) * 512],
                            in_=ob[:])
                tc.cur_priority = save

            for qc in range(NC):
                for hp in range(HP):
                    unit(hp, qc)
                final_proj(qc)

    nc.compile()
    return nc


_NC_CACHE = None


def _get_nc():
    global _NC_CACHE
    if _NC_CACHE is None:
        _NC_CACHE = build_nc()
    return _NC_CACHE


def make_in_maps(query, key_, value, mask, w_q, b_q, w_k, b_k, w_v, b_v,
                 w_o, b_o):
    import ml_dtypes
    f32 = np.float32
    bf16 = ml_dtypes.bfloat16
    c = lambda a: np.ascontiguousarray(a).astype(bf16)
    in_maps = []
    for core in range(N_CORES):
        b, g = core // 2, core % 2
        fs = slice(g * FL, (g + 1) * FL)
        mb = np.where(np.asarray(mask[b]).reshape(NT, 128).T,
                      0.0, -30.0).astype(f32)
        in_maps.append({
            "qT": c(query[b].T.astype(f32, copy=False)),
            "kT": c(key_[b].T.astype(f32, copy=False)),
            "vT": c(value[b].T.astype(f32, copy=False)),
            "wqT": c(w_q[fs, :].T.astype(f32, copy=False)),
            "wkT": c(w_k[fs, :].T.astype(f32, copy=False)),
            "wvT": c(w_v[fs, :].T.astype(f32, copy=False)),
            "woT": c(w_o[:, fs].T.astype(f32, copy=False)),
            "bqc": np.ascontiguousarray(
                b_q[fs].astype(f32, copy=False).reshape(HP, 128).T),
            "bkc": np.ascontiguousarray(
                b_k[fs].astype(f32, copy=False).reshape(HP, 128).T),
            "bvr": c(np.broadcast_to(b_v[fs], (128, FL))),
            "bor": c(np.broadcast_to(
                b_o if g == 0 else np.zeros(E, f32), (128, E))),
            "mbd": np.ascontiguousarray(mb),
        })
    return in_maps


def kernel(query=None, key_=None, value=None, mask=None, w_q=None, b_q=None,
           w_k=None, b_k=None, w_v=None, b_v=None, w_o=None, b_o=None,
           key=None, **_kwargs):
    if key_ is None:
        key_ = key
    args = [np.asarray(a) for a in
            (query, key_, value, mask, w_q, b_q, w_k, b_k, w_v, b_v,
             w_o, b_o)]
    nc = _get_nc()
    in_maps = make_in_maps(*args)
    res = run_bass_kernel_spmd(nc, in_maps, core_ids=list(range(N_CORES)))
    outs = [res.results[i]["out"] for i in range(N_CORES)]
    full = np.empty((B, T, E), np.float32)
    for b in range(B):
        full[b] = outs[2 * b] + outs[2 * b + 1]
    return full


# revision 7
# speedup vs baseline: 1.4075x; 1.2584x over previous
"""Multi-head attention Trainium2 Bass kernel (v2).

Shapes (hardcoded): B=4, T=2048, E=1024, H=16, DK=64.
Sharding over 8 cores: core c -> (batch b = c//2, head-group g = c%2).
Each core computes 8 heads of one batch end-to-end and a partial output
projection; the host sums the two partials per batch.

v2 design (ACT-saturation oriented; the exp stream is the roofline):
  - head-PAIR S matmuls: K/Q stored as natural f-tiles [128, T] where
    rows 0:64 = head 2i's dk and rows 64:128 = head 2i+1's dk. The S
    matmul for a key tile is a row-tiled PE pair (tile_position (0,0) /
    (64,0)) computing BOTH heads concurrently in one 512-cycle stream.
    No row duplication DMAs needed.
  - key mask applied as a per-partition bias operand of the exp
    activation (keys live on partitions of S^T): masked keys get
    bias=-30 => exp ~ 0, so they drop out of both attn@V and the
    row-sum column. V needs no masking.
  - every x chunk is loaded exactly once (f-loop inside chunk loop).
  - exp input tiles are [128, 1024] PSUM (2 banks), double buffered;
    attn@V accumulates per-head [65, 512] PSUM tiles (V plus a ones
    column producing the softmax row sums in row 64).
  - softmax normalization: row-sums bounce through DRAM to turn the
    [1, 512] sums row into [64, 16] lanes for the DVE reciprocal, then
    a stride-0 broadcast DMA replicates the reciprocals to 64
    partitions for the normalize multiply.
  - output projection per q-chunk with the bias added by the DVE
    (tensor_add with a pre-replicated bias tile) during the PSUM->SBUF
    move; no bias matmuls.
"""

import numpy as np

import concourse.bass as bass
import concourse.tile as tile
from concourse import bacc, mybir
from concourse.bass_utils import run_bass_kernel_spmd

F32 = mybir.dt.float32
BF16 = mybir.dt.bfloat16
DT = BF16

B, T, E, H = 4, 2048, 1024, 16
DK = E // H            # 64
N_CORES = 8
FL = 512               # local f (8 heads * 64)
HL = 8                 # heads per core
HP = HL // 2           # head pairs per core = f tiles
NT = T // 128          # 16 key tiles
NE = E // 128          # 8 e tiles
NC = T // 512          # 4 chunks of 512

# Priority classes (lower = preferred by the static Tile scheduler).
# The attention stream (S-pairs, exp, attn@V, norm) keeps its natural
# emission indices (~0..20k); projection and output-projection work is
# demoted far below so it only fills engine slack and never sits ahead
# of ready attention work in an engine's static FIFO.
PRI_PROJ = 1_000_000
PRI_FINAL = 2_000_000


def build_nc():
    nc = bacc.Bacc("TRN2", target_bir_lowering=False, debug=False,
                   enable_asserts=False)

    qT = nc.dram_tensor("qT", [E, T], DT, kind="ExternalInput").ap()
    kT = nc.dram_tensor("kT", [E, T], DT, kind="ExternalInput").ap()
    vT = nc.dram_tensor("vT", [E, T], DT, kind="ExternalInput").ap()
    wqT = nc.dram_tensor("wqT", [E, FL], DT, kind="ExternalInput").ap()
    wkT = nc.dram_tensor("wkT", [E, FL], DT, kind="ExternalInput").ap()
    wvT = nc.dram_tensor("wvT", [E, FL], DT, kind="ExternalInput").ap()
    woT = nc.dram_tensor("woT", [FL, E], DT, kind="ExternalInput").ap()
    bqc = nc.dram_tensor("bqc", [128, HP], F32, kind="ExternalInput").ap()
    bkc = nc.dram_tensor("bkc", [128, HP], F32, kind="ExternalInput").ap()
    bvr = nc.dram_tensor("bvr", [128, FL], DT, kind="ExternalInput").ap()
    bor = nc.dram_tensor("bor", [128, E], DT, kind="ExternalInput").ap()
    # per-key additive exp bias: 0 (allowed) or -30 (masked)
    mbd = nc.dram_tensor("mbd", [128, NT], F32, kind="ExternalInput").ap()
    out = nc.dram_tensor("out", [T, E], F32, kind="ExternalOutput").ap()

    with tile.TileContext(nc) as tc:
        with (
            tc.tile_pool(name="const", bufs=1) as constp,
            tc.tile_pool(name="qkt", bufs=1) as qktp,
            tc.tile_pool(name="vsb", bufs=1) as vsbp,
            tc.tile_pool(name="xtl", bufs=1) as xtlp,
            tc.tile_pool(name="wgt", bufs=1) as wp,
            tc.tile_pool(name="xk", bufs=2) as xkp,
            tc.tile_pool(name="xq", bufs=2) as xqp,
            tc.tile_pool(name="xv", bufs=2) as xvp,
            tc.tile_pool(name="es", bufs=8) as esp,
            tc.tile_pool(name="norm", bufs=2) as normp,
            tc.tile_pool(name="normd", bufs=2, space="DRAM") as normdp,
            tc.tile_pool(name="ob", bufs=4) as obp,
            tc.tile_pool(name="ps_s", bufs=2, space="PSUM") as ps_s,
            tc.tile_pool(name="ps_o", bufs=1, space="PSUM") as ps_o,
            tc.tile_pool(name="ps_w", bufs=2, space="PSUM") as ps_w,
        ):
            # ---- constants ----
            bq_sb = constp.tile([128, HP], F32, tag="bq")
            nc.sync.dma_start(out=bq_sb[:], in_=bqc)
            bk_sb = constp.tile([128, HP], F32, tag="bk")
            nc.sync.dma_start(out=bk_sb[:], in_=bkc)
            bv_sb = constp.tile([128, FL], DT, tag="bv")
            nc.sync.dma_start(out=bv_sb[:], in_=bvr)
            bo_sb = constp.tile([128, E], DT, tag="bo")
            nc.sync.dma_start(out=bo_sb[:], in_=bor)
            mb_sb = constp.tile([128, NT], F32, tag="mb")
            nc.sync.dma_start(out=mb_sb[:], in_=mbd)

            # persistent activations
            kt = [qktp.tile([128, T], DT, tag=f"kt{f}", name=f"kt{f}")
                  for f in range(HP)]
            qt = [qktp.tile([128, T], DT, tag=f"qt{f}", name=f"qt{f}")
                  for f in range(HP)]
            # V per key tile: [128 keys, 8 heads * 65]; per head cols
            # 0..63 = V, col 64 = ones (softmax row-sum trick)
            vt = [vsbp.tile([128, HL * 65], DT, tag=f"v{j}", name=f"v{j}")
                  for j in range(NT)]
            xtl = [xtlp.tile([128, T], DT, tag=f"x{f}", name=f"x{f}")
                   for f in range(HP)]
            for j in range(NT):
                nc.vector.memset(
                    vt[j].rearrange("p (h w) -> p h w", w=65)[:, :, 64:65],
                    1.0)

            # ---- weights ----
            wk_sb = [wp.tile([128, FL], DT, tag=f"wk{e}", name=f"wk{e}")
                     for e in range(NE)]
            wq_sb = [wp.tile([128, FL], DT, tag=f"wq{e}", name=f"wq{e}")
                     for e in range(NE)]
            wv_sb = [wp.tile([128, FL], DT, tag=f"wv{e}", name=f"wv{e}")
                     for e in range(NE)]
            wo_sb = [wp.tile([128, E], DT, tag=f"wo{f}", name=f"wo{f}")
                     for f in range(HP)]
            for e in range(NE):
                nc.sync.dma_start(out=wk_sb[e][:],
                                  in_=wkT[e * 128:(e + 1) * 128, :])
            for e in range(NE):
                nc.sync.dma_start(out=wq_sb[e][:],
                                  in_=wqT[e * 128:(e + 1) * 128, :])
            for e in range(NE):
                nc.sync.dma_start(out=wv_sb[e][:],
                                  in_=wvT[e * 128:(e + 1) * 128, :])
            for f in range(HP):
                nc.sync.dma_start(out=wo_sb[f][:],
                                  in_=woT[f * 128:(f + 1) * 128, :])

            def demote(off):
                save = tc.cur_priority
                tc.cur_priority = save + off
                return save

            def qk_proj(c, xdram, w_sb, bias_sb, dst, pri):
                save = demote(pri)
                xs = []
                for e in range(NE):
                    xe = (xkp if dst is kt else xqp).tile(
                        [128, 512], DT, tag=f"x{e}", name=f"x{e}")
                    nc.sync.dma_start(
                        out=xe[:],
                        in_=xdram[e * 128:(e + 1) * 128,
                                  c * 512:(c + 1) * 512])
                    xs.append(xe)
                for f in range(HP):
                    ps = ps_w.tile([128, 512], F32, tag="psw", name="psw")
                    for e in range(NE):
                        nc.tensor.matmul(
                            ps[:],
                            lhsT=w_sb[e][:, f * 128:(f + 1) * 128],
                            rhs=xs[e][:],
                            start=(e == 0), stop=(e == NE - 1))
                    nc.vector.tensor_scalar_add(
                        dst[f][:, c * 512:(c + 1) * 512],
                        ps[:], bias_sb[:, f:f + 1])
                tc.cur_priority = save

            def v_proj(c, pri):
                save = demote(pri)
                xs = []
                for e in range(NE):
                    xe = xvp.tile([128, 512], DT, tag=f"x{e}", name=f"x{e}")
                    nc.sync.dma_start(
                        out=xe[:],
                        in_=vT[e * 128:(e + 1) * 128,
                               c * 512:(c + 1) * 512])
                    xs.append(xe)
                for jj in range(4):
                    j = 4 * c + jj
                    ps = ps_w.tile([128, 512], F32, tag="psw", name="psw")
                    for e in range(NE):
                        nc.tensor.matmul(
                            ps[:],
                            lhsT=xs[e][:, jj * 128:(jj + 1) * 128],
                            rhs=wv_sb[e][:],
                            start=(e == 0), stop=(e == NE - 1))
                    nc.vector.tensor_add(
                        vt[j].rearrange("p (h w) -> p h w", w=65)[:, :, 0:64],
                        ps.rearrange("p (h w) -> p h w", w=64),
                        bv_sb.rearrange("p (h w) -> p h w", w=64))
                tc.cur_priority = save

            # ---- projections (all emitted up front, need-ordered) ----
            qk_proj(0, kT, wk_sb, bk_sb, kt, PRI_PROJ)
            qk_proj(0, qT, wq_sb, bq_sb, qt, PRI_PROJ)
            qk_proj(1, kT, wk_sb, bk_sb, kt, PRI_PROJ)
            v_proj(0, PRI_PROJ)
            qk_proj(2, kT, wk_sb, bk_sb, kt, PRI_PROJ)
            v_proj(1, PRI_PROJ)
            qk_proj(3, kT, wk_sb, bk_sb, kt, PRI_PROJ)
            v_proj(2, PRI_PROJ)
            v_proj(3, PRI_PROJ)
            qk_proj(1, qT, wq_sb, bq_sb, qt, PRI_PROJ)
            qk_proj(2, qT, wq_sb, bq_sb, qt, PRI_PROJ)
            qk_proj(3, qT, wq_sb, bq_sb, qt, PRI_PROJ)

            # ---- attention + output projection ----
            def unit(hp, qc):
                """One (head-pair, q-chunk-512) attention unit."""
                qsl = slice(qc * 512, (qc + 1) * 512)
                psoA = ps_o.tile([65, 512], F32, tag="psoA", name="psoA")
                psoB = ps_o.tile([65, 512], F32, tag="psoB", name="psoB")
                for k in range(NT):
                    pss = ps_s.tile([128, 1024], F32, tag="pss",
                                    name="pss")
                    for t in range(2):
                        r = slice(t * 64, (t + 1) * 64)
                        nc.tensor.matmul(
                            pss[:, t * 512:(t + 1) * 512],
                            lhsT=kt[hp][r, k * 128:(k + 1) * 128],
                            rhs=qt[hp][r, qsl],
                            start=True, stop=True,
                            tile_position=(t * 64, 0))
                    es = esp.tile([128, 1024], DT, tag="es", name="es")
                    nc.scalar.activation(
                        out=es[:], in_=pss[:],
                        func=mybir.ActivationFunctionType.Exp,
                        bias=mb_sb[:, k:k + 1], scale=0.125)
                    h0 = 2 * hp
                    nc.tensor.matmul(
                        psoA[:], lhsT=vt[k][:, h0 * 65:(h0 + 1) * 65],
                        rhs=es[:, 0:512],
                        start=(k == 0), stop=(k == NT - 1))
                    nc.tensor.matmul(
                        psoB[:], lhsT=vt[k][:, (h0 + 1) * 65:(h0 + 2) * 65],
                        rhs=es[:, 512:1024],
                        start=(k == 0), stop=(k == NT - 1))
                # Copy the accumulators to SBUF right away so the single
                # pso PSUM buffer is freed for the next unit; the whole
                # normalization chain then runs from SBUF off the PSUM
                # critical path. Rows 0..63 = O^T, row 64 = sum(exp).
                otA = normp.tile([65, 512], F32, tag="otA", name="otA")
                nc.vector.tensor_copy(out=otA[:], in_=psoA[:])
                otB = normp.tile([65, 512], F32, tag="otB", name="otB")
                nc.vector.tensor_copy(out=otB[:], in_=psoB[:])
                rsd = normdp.tile([1, 1024], F32, tag="rsd", name="rsd")
                nc.sync.dma_start(out=rsd[:, 0:512], in_=otA[64:65, :])
                nc.sync.dma_start(out=rsd[:, 512:1024], in_=otB[64:65, :])
                rs = normp.tile([64, 16], F32, tag="rs", name="rs")
                nc.sync.dma_start(
                    out=rs[:],
                    in_=rsd.rearrange("o (p w) -> (o p) w", w=16))
                ri = normp.tile([64, 16], F32, tag="ri", name="ri")
                nc.vector.reciprocal(ri[:], rs[:])
                rid = normdp.tile([64, 16], F32, tag="rid", name="rid")
                nc.sync.dma_start(out=rid[:], in_=ri[:])
                rif = rid.rearrange("p w -> () (p w)")
                repA = normp.tile([64, 512], F32, tag="repA", name="repA")
                nc.sync.dma_start(out=repA[:],
                                  in_=rif[:, 0:512].to_broadcast([64, 512]))
                repB = normp.tile([64, 512], F32, tag="repB", name="repB")
                nc.sync.dma_start(out=repB[:],
                                  in_=rif[:, 512:1024].to_broadcast([64, 512]))
                nc.vector.tensor_mul(
                    xtl[hp][0:64, qsl], otA[0:64, :], repA[:])
                nc.vector.tensor_mul(
                    xtl[hp][64:128, qsl], otB[0:64, :], repB[:])

            def final_group(j, c2):
                """Output projection for q rows j*128.. and E half c2."""
                save = demote(PRI_FINAL)
                ps = ps_w.tile([128, 512], F32, tag="psw", name="psf")
                for f in range(HP):
                    nc.tensor.matmul(
                        ps[:],
                        lhsT=xtl[f][:, j * 128:(j + 1) * 128],
                        rhs=wo_sb[f][:, c2 * 512:(c2 + 1) * 512],
                        start=(f == 0), stop=(f == HP - 1))
                ob = obp.tile([128, 512], F32, tag="ob", name="ob")
                nc.vector.tensor_add(
                    ob[:], ps[:], bo_sb[:, c2 * 512:(c2 + 1) * 512])
                nc.sync.dma_start(
                    out=out[j * 128:(j + 1) * 128,
                            c2 * 512:(c2 + 1) * 512],
                    in_=ob[:])
                tc.cur_priority = save

            # Emit each qc's output projection interleaved into the NEXT
            # qc's units so its (demoted) instructions never sit at a
            # unit boundary waiting on the normalization chain.
            pending = []
            for qc in range(NC):
                for hp in range(HP):
                    unit(hp, qc)
                    for _ in range(2):
                        if pending:
                            final_group(*pending.pop(0))
                pending = [(j, c2) for j in range(4 * qc, 4 * qc + 4)
                           for c2 in range(2)]
            for grp in pending:
                final_group(*grp)

    nc.compile()
    return nc


_NC_CACHE = None


def _get_nc():
    global _NC_CACHE
    if _NC_CACHE is None:
        _NC_CACHE = build_nc()
    return _NC_CACHE


def make_in_maps(query, key_, value, mask, w_q, b_q, w_k, b_k, w_v, b_v,
                 w_o, b_o):
    import ml_dtypes
    f32 = np.float32
    bf16 = ml_dtypes.bfloat16
    c = lambda a: np.ascontiguousarray(a).astype(bf16)
    in_maps = []
    for core in range(N_CORES):
        b, g = core // 2, core % 2
        fs = slice(g * FL, (g + 1) * FL)
        mb = np.where(np.asarray(mask[b]).reshape(NT, 128).T,
                      0.0, -30.0).astype(f32)
        in_maps.append({
            "qT": c(query[b].T.astype(f32, copy=False)),
            "kT": c(key_[b].T.astype(f32, copy=False)),
            "vT": c(value[b].T.astype(f32, copy=False)),
            "wqT": c(w_q[fs, :].T.astype(f32, copy=False)),
            "wkT": c(w_k[fs, :].T.astype(f32, copy=False)),
            "wvT": c(w_v[fs, :].T.astype(f32, copy=False)),
            "woT": c(w_o[:, fs].T.astype(f32, copy=False)),
            "bqc": np.ascontiguousarray(
                b_q[fs].astype(f32, copy=False).reshape(HP, 128).T),
            "bkc": np.ascontiguousarray(
                b_k[fs].astype(f32, copy=False).reshape(HP, 128).T),
            "bvr": c(np.broadcast_to(b_v[fs], (128, FL))),
            "bor": c(np.broadcast_to(
                b_o if g == 0 else np.zeros(E, f32), (128, E))),
            "mbd": np.ascontiguousarray(mb),
        })
    return in_maps


def kernel(query=None, key_=None, value=None, mask=None, w_q=None, b_q=None,
           w_k=None, b_k=None, w_v=None, b_v=None, w_o=None, b_o=None,
           key=None, **_kwargs):
    if key_ is None:
        key_ = key
    args = [np.asarray(a) for a in
            (query, key_, value, mask, w_q, b_q, w_k, b_k, w_v, b_v,
             w_o, b_o)]
    nc = _get_nc()
    in_maps = make_in_maps(*args)
    res = run_bass_kernel_spmd(nc, in_maps, core_ids=list(range(N_CORES)))
    outs = [res.results[i]["out"] for i in range(N_CORES)]
    full = np.empty((B, T, E), np.float32)
    for b in range(B):
        full[b] = outs[2 * b] + outs[2 * b + 1]
    return full


# revision 10
# speedup vs baseline: 1.4999x; 1.0656x over previous
"""Multi-head attention Trainium2 Bass kernel (v2).

Shapes (hardcoded): B=4, T=2048, E=1024, H=16, DK=64.
Sharding over 8 cores: core c -> (batch b = c//2, head-group g = c%2).
Each core computes 8 heads of one batch end-to-end and a partial output
projection; the host sums the two partials per batch.

v2 design (ACT-saturation oriented; the exp stream is the roofline):
  - head-PAIR S matmuls: K/Q stored as natural f-tiles [128, T] where
    rows 0:64 = head 2i's dk and rows 64:128 = head 2i+1's dk. The S
    matmul for a key tile is a row-tiled PE pair (tile_position (0,0) /
    (64,0)) computing BOTH heads concurrently in one 512-cycle stream.
    No row duplication DMAs needed.
  - key mask applied as a per-partition bias operand of the exp
    activation (keys live on partitions of S^T): masked keys get
    bias=-30 => exp ~ 0, so they drop out of both attn@V and the
    row-sum column. V needs no masking.
  - every x chunk is loaded exactly once (f-loop inside chunk loop).
  - exp input tiles are [128, 1024] PSUM (2 banks), double buffered;
    attn@V accumulates per-head [65, 512] PSUM tiles (V plus a ones
    column producing the softmax row sums in row 64).
  - softmax normalization: row-sums bounce through DRAM to turn the
    [1, 512] sums row into [64, 16] lanes for the DVE reciprocal, then
    a stride-0 broadcast DMA replicates the reciprocals to 64
    partitions for the normalize multiply.
  - output projection per q-chunk with the bias added by the DVE
    (tensor_add with a pre-replicated bias tile) during the PSUM->SBUF
    move; no bias matmuls.
"""

import numpy as np

import concourse.bass as bass
import concourse.tile as tile
from concourse import bacc, mybir
from concourse.bass_utils import run_bass_kernel_spmd

F32 = mybir.dt.float32
BF16 = mybir.dt.bfloat16
DT = BF16

B, T, E, H = 4, 2048, 1024, 16
DK = E // H            # 64
N_CORES = 8
FL = 512               # local f (8 heads * 64)
HL = 8                 # heads per core
HP = HL // 2           # head pairs per core = f tiles
NT = T // 128          # 16 key tiles
NE = E // 128          # 8 e tiles
NC = T // 512          # 4 chunks of 512

# Priority classes (lower = preferred by the static Tile scheduler).
# The attention stream (S-pairs, exp, attn@V, norm) keeps its natural
# emission indices (~0..20k); projection and output-projection work is
# demoted far below so it only fills engine slack and never sits ahead
# of ready attention work in an engine's static FIFO.
PRI_PROJ = 1_000_000
PRI_FINAL = 2_000_000


def build_nc():
    nc = bacc.Bacc("TRN2", target_bir_lowering=False, debug=False,
                   enable_asserts=False)

    qT = nc.dram_tensor("qT", [E, T], DT, kind="ExternalInput").ap()
    kT = nc.dram_tensor("kT", [E, T], DT, kind="ExternalInput").ap()
    vT = nc.dram_tensor("vT", [E, T], DT, kind="ExternalInput").ap()
    wqT = nc.dram_tensor("wqT", [E, FL], DT, kind="ExternalInput").ap()
    wkT = nc.dram_tensor("wkT", [E, FL], DT, kind="ExternalInput").ap()
    wvT = nc.dram_tensor("wvT", [E, FL], DT, kind="ExternalInput").ap()
    woT = nc.dram_tensor("woT", [FL, E], DT, kind="ExternalInput").ap()
    bqc = nc.dram_tensor("bqc", [128, HP], F32, kind="ExternalInput").ap()
    bkc = nc.dram_tensor("bkc", [128, HP], F32, kind="ExternalInput").ap()
    bvr = nc.dram_tensor("bvr", [128, FL], DT, kind="ExternalInput").ap()
    bor = nc.dram_tensor("bor", [128, E], DT, kind="ExternalInput").ap()
    # per-key additive exp bias: 0 (allowed) or -30 (masked)
    mbd = nc.dram_tensor("mbd", [128, NT], F32, kind="ExternalInput").ap()
    out = nc.dram_tensor("out", [T, E], F32, kind="ExternalOutput").ap()

    with tile.TileContext(nc) as tc:
        with (
            tc.tile_pool(name="const", bufs=1) as constp,
            tc.tile_pool(name="qkt", bufs=1) as qktp,
            tc.tile_pool(name="vsb", bufs=1) as vsbp,
            tc.tile_pool(name="xtl", bufs=1) as xtlp,
            tc.tile_pool(name="wgt", bufs=1) as wp,
            tc.tile_pool(name="xk", bufs=2) as xkp,
            tc.tile_pool(name="xq", bufs=2) as xqp,
            tc.tile_pool(name="xv", bufs=2) as xvp,
            tc.tile_pool(name="es", bufs=8) as esp,
            tc.tile_pool(name="norm", bufs=2) as normp,
            tc.tile_pool(name="normd", bufs=2, space="DRAM") as normdp,
            tc.tile_pool(name="ob", bufs=4) as obp,
            tc.tile_pool(name="ps_s", bufs=2, space="PSUM") as ps_s,
            tc.tile_pool(name="ps_o", bufs=1, space="PSUM") as ps_o,
            tc.tile_pool(name="ps_w", bufs=2, space="PSUM") as ps_w,
        ):
            # ---- constants ----
            bq_sb = constp.tile([128, HP], F32, tag="bq")
            nc.sync.dma_start(out=bq_sb[:], in_=bqc)
            bk_sb = constp.tile([128, HP], F32, tag="bk")
            nc.sync.dma_start(out=bk_sb[:], in_=bkc)
            bv_sb = constp.tile([128, FL], DT, tag="bv")
            nc.sync.dma_start(out=bv_sb[:], in_=bvr)
            bo_sb = constp.tile([128, E], DT, tag="bo")
            nc.sync.dma_start(out=bo_sb[:], in_=bor)
            mb_sb = constp.tile([128, NT], F32, tag="mb")
            nc.sync.dma_start(out=mb_sb[:], in_=mbd)

            # persistent activations
            kt = [qktp.tile([128, T], DT, tag=f"kt{f}", name=f"kt{f}")
                  for f in range(HP)]
            qt = [qktp.tile([128, T], DT, tag=f"qt{f}", name=f"qt{f}")
                  for f in range(HP)]
            # V per key tile: [128 keys, 8 heads * 65]; per head cols
            # 0..63 = V, col 64 = ones (softmax row-sum trick)
            vt = [vsbp.tile([128, HL * 65], DT, tag=f"v{j}", name=f"v{j}")
                  for j in range(NT)]
            xtl = [xtlp.tile([128, T], DT, tag=f"x{f}", name=f"x{f}")
                   for f in range(HP)]
            for j in range(NT):
                nc.vector.memset(
                    vt[j].rearrange("p (h w) -> p h w", w=65)[:, :, 64:65],
                    1.0)

            # ---- weights (one batched DMA per tensor) ----
            # wk_sb [128, e*512 + f_cols]: e-tile e lives at cols
            # e*512..(e+1)*512
            wk_sb = wp.tile([128, NE * FL], DT, tag="wk", name="wk")
            nc.sync.dma_start(
                out=wk_sb.rearrange("p (e f) -> p e f", f=FL),
                in_=wkT.rearrange("(e p) f -> p e f", p=128))
            wq_sb = wp.tile([128, NE * FL], DT, tag="wq", name="wq")
            nc.sync.dma_start(
                out=wq_sb.rearrange("p (e f) -> p e f", f=FL),
                in_=wqT.rearrange("(e p) f -> p e f", p=128))
            wv_sb = wp.tile([128, NE * FL], DT, tag="wv", name="wv")
            nc.sync.dma_start(
                out=wv_sb.rearrange("p (e f) -> p e f", f=FL),
                in_=wvT.rearrange("(e p) f -> p e f", p=128))
            wo_sb = wp.tile([128, HP * E], DT, tag="wo", name="wo")
            nc.sync.dma_start(
                out=wo_sb.rearrange("p (f e) -> p f e", e=E),
                in_=woT.rearrange("(f p) e -> p f e", p=128))

            def demote(off):
                save = tc.cur_priority
                tc.cur_priority = save + off
                return save

            qT3 = qT.rearrange("(e p) t -> p e t", p=128)
            kT3 = kT.rearrange("(e p) t -> p e t", p=128)
            vT3 = vT.rearrange("(e p) t -> p e t", p=128)

            def load_chunk(pool, xdram3, c):
                """One DMA for a [all-e, 512-q] chunk of an input."""
                xe = pool.tile([128, NE * 512], DT, tag="x", name="x")
                nc.sync.dma_start(
                    out=xe.rearrange("p (e q) -> p e q", q=512),
                    in_=xdram3[:, :, c * 512:(c + 1) * 512])
                return xe

            def qk_proj(c, xe, w_sb, bias_sb, dst, fl, pri):
                save = demote(pri)
                for f in fl:
                    ps = ps_w.tile([128, 512], F32, tag="psw", name="psw")
                    for e in range(NE):
                        nc.tensor.matmul(
                            ps[:],
                            lhsT=w_sb[:, e * FL + f * 128:
                                      e * FL + (f + 1) * 128],
                            rhs=xe[:, e * 512:(e + 1) * 512],
                            start=(e == 0), stop=(e == NE - 1))
                    nc.vector.tensor_scalar_add(
                        dst[f][:, c * 512:(c + 1) * 512],
                        ps[:], bias_sb[:, f:f + 1])
                tc.cur_priority = save

            def v_proj(c, xe, pri):
                save = demote(pri)
                for jj in range(4):
                    j = 4 * c + jj
                    ps = ps_w.tile([128, 512], F32, tag="psw", name="psw")
                    for e in range(NE):
                        nc.tensor.matmul(
                            ps[:],
                            lhsT=xe[:, e * 512 + jj * 128:
                                    e * 512 + (jj + 1) * 128],
                            rhs=wv_sb[:, e * FL:(e + 1) * FL],
                            start=(e == 0), stop=(e == NE - 1))
                    nc.vector.tensor_add(
                        vt[j].rearrange("p (h w) -> p h w", w=65)[:, :, 0:64],
                        ps.rearrange("p (h w) -> p h w", w=64),
                        bv_sb.rearrange("p (h w) -> p h w", w=64))
                tc.cur_priority = save

            # ---- projections (all emitted up front, need-ordered).
            # K/Q f0 of chunk 0 first so the first S-pair unblocks after
            # ~16 matmuls instead of 64.
            xk0 = load_chunk(xkp, kT3, 0)
            xq0 = load_chunk(xqp, qT3, 0)
            qk_proj(0, xk0, wk_sb, bk_sb, kt, [0], PRI_PROJ)
            qk_proj(0, xq0, wq_sb, bq_sb, qt, [0], PRI_PROJ)
            qk_proj(0, xk0, wk_sb, bk_sb, kt, [1, 2, 3], PRI_PROJ)
            qk_proj(0, xq0, wq_sb, bq_sb, qt, [1, 2, 3], PRI_PROJ)
            xk1 = load_chunk(xkp, kT3, 1)
            qk_proj(1, xk1, wk_sb, bk_sb, kt, [0, 1, 2, 3], PRI_PROJ)
            xv0 = load_chunk(xvp, vT3, 0)
            v_proj(0, xv0, PRI_PROJ)
            xk2 = load_chunk(xkp, kT3, 2)
            qk_proj(2, xk2, wk_sb, bk_sb, kt, [0, 1, 2, 3], PRI_PROJ)
            xv1 = load_chunk(xvp, vT3, 1)
            v_proj(1, xv1, PRI_PROJ)
            xk3 = load_chunk(xkp, kT3, 3)
            qk_proj(3, xk3, wk_sb, bk_sb, kt, [0, 1, 2, 3], PRI_PROJ)
            xv2 = load_chunk(xvp, vT3, 2)
            v_proj(2, xv2, PRI_PROJ)
            xv3 = load_chunk(xvp, vT3, 3)
            v_proj(3, xv3, PRI_PROJ)
            for c in range(1, NC):
                xq = load_chunk(xqp, qT3, c)
                qk_proj(c, xq, wq_sb, bq_sb, qt, [0, 1, 2, 3], PRI_PROJ)

            # ---- attention + output projection ----
            def unit(hp, qc):
                """One (head-pair, q-chunk-512) attention unit."""
                qsl = slice(qc * 512, (qc + 1) * 512)
                psoA = ps_o.tile([65, 512], F32, tag="psoA", name="psoA")
                psoB = ps_o.tile([65, 512], F32, tag="psoB", name="psoB")
                for k in range(NT):
                    pss = ps_s.tile([128, 1024], F32, tag="pss",
                                    name="pss")
                    for t in range(2):
                        r = slice(t * 64, (t + 1) * 64)
                        nc.tensor.matmul(
                            pss[:, t * 512:(t + 1) * 512],
                            lhsT=kt[hp][r, k * 128:(k + 1) * 128],
                            rhs=qt[hp][r, qsl],
                            start=True, stop=True,
                            tile_position=(t * 64, 0))
                    es = esp.tile([128, 1024], DT, tag="es", name="es")
                    nc.scalar.activation(
                        out=es[:], in_=pss[:],
                        func=mybir.ActivationFunctionType.Exp,
                        bias=mb_sb[:, k:k + 1], scale=0.125)
                    h0 = 2 * hp
                    nc.tensor.matmul(
                        psoA[:], lhsT=vt[k][:, h0 * 65:(h0 + 1) * 65],
                        rhs=es[:, 0:512],
                        start=(k == 0), stop=(k == NT - 1))
                    nc.tensor.matmul(
                        psoB[:], lhsT=vt[k][:, (h0 + 1) * 65:(h0 + 2) * 65],
                        rhs=es[:, 512:1024],
                        start=(k == 0), stop=(k == NT - 1))
                # Copy the accumulators to SBUF right away so the single
                # pso PSUM buffer is freed for the next unit; the whole
                # normalization chain then runs from SBUF off the PSUM
                # critical path. Rows 0..63 = O^T, row 64 = sum(exp).
                ot = normp.tile([65, 1024], F32, tag="ot", name="ot")
                nc.vector.tensor_copy(out=ot[:, 0:512], in_=psoA[:])
                nc.vector.tensor_copy(out=ot[:, 512:1024], in_=psoB[:])
                rsd = normdp.tile([1, 1024], F32, tag="rsd", name="rsd")
                nc.sync.dma_start(out=rsd[:], in_=ot[64:65, :])
                rs = normp.tile([64, 16], F32, tag="rs", name="rs")
                nc.sync.dma_start(
                    out=rs[:],
                    in_=rsd.rearrange("o (p w) -> (o p) w", w=16))
                ri = normp.tile([64, 16], F32, tag="ri", name="ri")
                nc.vector.reciprocal(ri[:], rs[:])
                rid = normdp.tile([64, 16], F32, tag="rid", name="rid")
                nc.sync.dma_start(out=rid[:], in_=ri[:])
                rep = normp.tile([64, 1024], F32, tag="rep", name="rep")
                nc.sync.dma_start(
                    out=rep[:],
                    in_=rid.rearrange("p w -> () (p w)").to_broadcast(
                        [64, 1024]))
                nc.vector.tensor_mul(
                    xtl[hp][0:64, qsl], ot[0:64, 0:512], rep[:, 0:512])
                nc.vector.tensor_mul(
                    xtl[hp][64:128, qsl], ot[0:64, 512:1024],
                    rep[:, 512:1024])

            def final_group(j, c2):
                """Output projection for q rows j*128.. and E half c2."""
                save = demote(PRI_FINAL)
                ps = ps_w.tile([128, 512], F32, tag="psw", name="psf")
                for f in range(HP):
                    nc.tensor.matmul(
                        ps[:],
                        lhsT=xtl[f][:, j * 128:(j + 1) * 128],
                        rhs=wo_sb[:, f * E + c2 * 512:
                                  f * E + (c2 + 1) * 512],
                        start=(f == 0), stop=(f == HP - 1))
                ob = obp.tile([128, 512], F32, tag="ob", name="ob")
                nc.vector.tensor_add(
                    ob[:], ps[:], bo_sb[:, c2 * 512:(c2 + 1) * 512])
                nc.sync.dma_start(
                    out=out[j * 128:(j + 1) * 128,
                            c2 * 512:(c2 + 1) * 512],
                    in_=ob[:])
                tc.cur_priority = save

            # Emit each qc's output projection interleaved into the NEXT
            # qc's units so its (demoted) instructions never sit at a
            # unit boundary waiting on the normalization chain.
            pending = []
            for qc in range(NC):
                for hp in range(HP):
                    unit(hp, qc)
                    for _ in range(2):
                        if pending:
                            final_group(*pending.pop(0))
                pending = [(j, c2) for j in range(4 * qc, 4 * qc + 4)
                           for c2 in range(2)]
            for grp in pending:
                final_group(*grp)

    nc.compile()
    return nc


_NC_CACHE = None


def _get_nc():
    global _NC_CACHE
    if _NC_CACHE is None:
        _NC_CACHE = build_nc()
    return _NC_CACHE


def make_in_maps(query, key_, value, mask, w_q, b_q, w_k, b_k, w_v, b_v,
                 w_o, b_o):
    import ml_dtypes
    f32 = np.float32
    bf16 = ml_dtypes.bfloat16
    c = lambda a: np.ascontiguousarray(a).astype(bf16)
    in_maps = []
    for core in range(N_CORES):
        b, g = core // 2, core % 2
        fs = slice(g * FL, (g + 1) * FL)
        mb = np.where(np.asarray(mask[b]).reshape(NT, 128).T,
                      0.0, -30.0).astype(f32)
        in_maps.append({
            "qT": c(query[b].T.astype(f32, copy=False)),
            "kT": c(key_[b].T.astype(f32, copy=False)),
            "vT": c(value[b].T.astype(f32, copy=False)),
            "wqT": c(w_q[fs, :].T.astype(f32, copy=False)),
            "wkT": c(w_k[fs, :].T.astype(f32, copy=False)),
            "wvT": c(w_v[fs, :].T.astype(f32, copy=False)),
            "woT": c(w_o[:, fs].T.astype(f32, copy=False)),
            "bqc": np.ascontiguousarray(
                b_q[fs].astype(f32, copy=False).reshape(HP, 128).T),
            "bkc": np.ascontiguousarray(
                b_k[fs].astype(f32, copy=False).reshape(HP, 128).T),
            "bvr": c(np.broadcast_to(b_v[fs], (128, FL))),
            "bor": c(np.broadcast_to(
                b_o if g == 0 else np.zeros(E, f32), (128, E))),
            "mbd": np.ascontiguousarray(mb),
        })
    return in_maps


def kernel(query=None, key_=None, value=None, mask=None, w_q=None, b_q=None,
           w_k=None, b_k=None, w_v=None, b_v=None, w_o=None, b_o=None,
           key=None, **_kwargs):
    if key_ is None:
        key_ = key
    args = [np.asarray(a) for a in
            (query, key_, value, mask, w_q, b_q, w_k, b_k, w_v, b_v,
             w_o, b_o)]
    nc = _get_nc()
    in_maps = make_in_maps(*args)
    res = run_bass_kernel_spmd(nc, in_maps, core_ids=list(range(N_CORES)))
    outs = [res.results[i]["out"] for i in range(N_CORES)]
    full = np.empty((B, T, E), np.float32)
    for b in range(B):
        full[b] = outs[2 * b] + outs[2 * b + 1]
    return full


# revision 17
# speedup vs baseline: 1.5027x; 1.0018x over previous
"""Multi-head attention Trainium2 Bass kernel (v2).

Shapes (hardcoded): B=4, T=2048, E=1024, H=16, DK=64.
Sharding over 8 cores: core c -> (batch b = c//2, head-group g = c%2).
Each core computes 8 heads of one batch end-to-end and a partial output
projection; the host sums the two partials per batch.

v2 design (ACT-saturation oriented; the exp stream is the roofline):
  - head-PAIR S matmuls: K/Q stored as natural f-tiles [128, T] where
    rows 0:64 = head 2i's dk and rows 64:128 = head 2i+1's dk. The S
    matmul for a key tile is a row-tiled PE pair (tile_position (0,0) /
    (64,0)) computing BOTH heads concurrently in one 512-cycle stream.
    No row duplication DMAs needed.
  - key mask applied as a per-partition bias operand of the exp
    activation (keys live on partitions of S^T): masked keys get
    bias=-30 => exp ~ 0, so they drop out of both attn@V and the
    row-sum column. V needs no masking.
  - every x chunk is loaded exactly once (f-loop inside chunk loop).
  - exp input tiles are [128, 1024] PSUM (2 banks), double buffered;
    attn@V accumulates per-head [65, 512] PSUM tiles (V plus a ones
    column producing the softmax row sums in row 64).
  - softmax normalization: row-sums bounce through DRAM to turn the
    [1, 512] sums row into [64, 16] lanes for the DVE reciprocal, then
    a stride-0 broadcast DMA replicates the reciprocals to 64
    partitions for the normalize multiply.
  - output projection per q-chunk with the bias added by the DVE
    (tensor_add with a pre-replicated bias tile) during the PSUM->SBUF
    move; no bias matmuls.
"""

import numpy as np

import concourse.bass as bass
import concourse.tile as tile
from concourse import bacc, mybir
from concourse.bass_utils import run_bass_kernel_spmd

F32 = mybir.dt.float32
BF16 = mybir.dt.bfloat16
DT = BF16

B, T, E, H = 4, 2048, 1024, 16
DK = E // H            # 64
N_CORES = 8
FL = 512               # local f (8 heads * 64)
HL = 8                 # heads per core
HP = HL // 2           # head pairs per core = f tiles
NT = T // 128          # 16 key tiles
NE = E // 128          # 8 e tiles
NC = T // 512          # 4 chunks of 512

# Priority classes (lower = preferred by the static Tile scheduler).
# The attention stream (S-pairs, exp, attn@V, norm) keeps its natural
# emission indices (~0..20k); projection and output-projection work is
# demoted far below so it only fills engine slack and never sits ahead
# of ready attention work in an engine's static FIFO.
PRI_NORM = 500_000
PRI_PROJ = 1_000_000
PRI_FINAL = 2_000_000


def build_nc():
    nc = bacc.Bacc("TRN2", target_bir_lowering=False, debug=False,
                   enable_asserts=False)

    qT = nc.dram_tensor("qT", [E, T], DT, kind="ExternalInput").ap()
    kT = nc.dram_tensor("kT", [E, T], DT, kind="ExternalInput").ap()
    vT = nc.dram_tensor("vT", [E, T], DT, kind="ExternalInput").ap()
    wqT = nc.dram_tensor("wqT", [E, FL], DT, kind="ExternalInput").ap()
    wkT = nc.dram_tensor("wkT", [E, FL], DT, kind="ExternalInput").ap()
    wvT = nc.dram_tensor("wvT", [E, FL], DT, kind="ExternalInput").ap()
    woT = nc.dram_tensor("woT", [FL, E], DT, kind="ExternalInput").ap()
    bqc = nc.dram_tensor("bqc", [128, HP], F32, kind="ExternalInput").ap()
    bkc = nc.dram_tensor("bkc", [128, HP], F32, kind="ExternalInput").ap()
    bvr = nc.dram_tensor("bvr", [128, FL], DT, kind="ExternalInput").ap()
    bor = nc.dram_tensor("bor", [128, E], DT, kind="ExternalInput").ap()
    # per-key additive exp bias: 0 (allowed) or -30 (masked)
    mbd = nc.dram_tensor("mbd", [128, NT], F32, kind="ExternalInput").ap()
    out = nc.dram_tensor("out", [T, E], F32, kind="ExternalOutput").ap()

    with tile.TileContext(nc) as tc:
        with (
            tc.tile_pool(name="const", bufs=1) as constp,
            tc.tile_pool(name="qkt", bufs=1) as qktp,
            tc.tile_pool(name="vsb", bufs=1) as vsbp,
            tc.tile_pool(name="xtl", bufs=1) as xtlp,
            tc.tile_pool(name="wgt", bufs=1) as wp,
            tc.tile_pool(name="xk", bufs=2) as xkp,
            tc.tile_pool(name="xq", bufs=2) as xqp,
            tc.tile_pool(name="xv", bufs=2) as xvp,
            tc.tile_pool(name="es", bufs=12) as esp,
            tc.tile_pool(name="norm", bufs=2) as normp,
            tc.tile_pool(name="normd", bufs=2, space="DRAM") as normdp,
            tc.tile_pool(name="ob", bufs=4) as obp,
            tc.tile_pool(name="ps_s", bufs=2, space="PSUM") as ps_s,
            tc.tile_pool(name="ps_o", bufs=1, space="PSUM") as ps_o,
            tc.tile_pool(name="ps_w", bufs=2, space="PSUM") as ps_w,
        ):
            # ---- constants ----
            bq_sb = constp.tile([128, HP], F32, tag="bq")
            nc.sync.dma_start(out=bq_sb[:], in_=bqc)
            bk_sb = constp.tile([128, HP], F32, tag="bk")
            nc.sync.dma_start(out=bk_sb[:], in_=bkc)
            bv_sb = constp.tile([128, FL], DT, tag="bv")
            nc.sync.dma_start(out=bv_sb[:], in_=bvr)
            bo_sb = constp.tile([128, E], DT, tag="bo")
            nc.sync.dma_start(out=bo_sb[:], in_=bor)
            mb_sb = constp.tile([128, NT], F32, tag="mb")
            nc.sync.dma_start(out=mb_sb[:], in_=mbd)

            # persistent activations
            kt = [qktp.tile([128, T], DT, tag=f"kt{f}", name=f"kt{f}")
                  for f in range(HP)]
            qt = [qktp.tile([128, T], DT, tag=f"qt{f}", name=f"qt{f}")
                  for f in range(HP)]
            # V per key tile: [128 keys, 8 heads * 65]; per head cols
            # 0..63 = V, col 64 = ones (softmax row-sum trick)
            vt = [vsbp.tile([128, HL * 65], DT, tag=f"v{j}", name=f"v{j}")
                  for j in range(NT)]
            xtl = [xtlp.tile([128, T], DT, tag=f"x{f}", name=f"x{f}")
                   for f in range(HP)]
            for j in range(NT):
                nc.vector.memset(
                    vt[j].rearrange("p (h w) -> p h w", w=65)[:, :, 64:65],
                    1.0)

            # ---- weights (one batched DMA per tensor; DMAs issued in
            # need-order below so the first K/Q chunks aren't queued
            # behind 3.5MB of weights) ----
            # wk_sb [128, e*512 + f_cols]: e-tile e lives at cols
            # e*512..(e+1)*512
            wk_sb = wp.tile([128, NE * FL], DT, tag="wk", name="wk")
            wq_sb = wp.tile([128, NE * FL], DT, tag="wq", name="wq")
            wv_sb = wp.tile([128, NE * FL], DT, tag="wv", name="wv")
            wo_sb = wp.tile([128, HP * E], DT, tag="wo", name="wo")

            def demote(off):
                save = tc.cur_priority
                tc.cur_priority = save + off
                return save

            qT3 = qT.rearrange("(e p) t -> p e t", p=128)
            kT3 = kT.rearrange("(e p) t -> p e t", p=128)
            vT3 = vT.rearrange("(e p) t -> p e t", p=128)

            def load_chunk(pool, xdram3, c):
                """One DMA for a [all-e, 512-q] chunk of an input."""
                xe = pool.tile([128, NE * 512], DT, tag="x", name="x")
                nc.sync.dma_start(
                    out=xe.rearrange("p (e q) -> p e q", q=512),
                    in_=xdram3[:, :, c * 512:(c + 1) * 512])
                return xe

            def qk_proj(c, xe, w_sb, bias_sb, dst, fl, pri):
                save = demote(pri)
                for f in fl:
                    ps = ps_w.tile([128, 512], F32, tag="psw", name="psw")
                    for e in range(NE):
                        nc.tensor.matmul(
                            ps[:],
                            lhsT=w_sb[:, e * FL + f * 128:
                                      e * FL + (f + 1) * 128],
                            rhs=xe[:, e * 512:(e + 1) * 512],
                            start=(e == 0), stop=(e == NE - 1))
                    nc.vector.tensor_scalar_add(
                        dst[f][:, c * 512:(c + 1) * 512],
                        ps[:], bias_sb[:, f:f + 1])
                tc.cur_priority = save

            def v_proj(c, xe, pri):
                save = demote(pri)
                for jj in range(4):
                    j = 4 * c + jj
                    ps = ps_w.tile([128, 512], F32, tag="psw", name="psw")
                    for e in range(NE):
                        nc.tensor.matmul(
                            ps[:],
                            lhsT=xe[:, e * 512 + jj * 128:
                                    e * 512 + (jj + 1) * 128],
                            rhs=wv_sb[:, e * FL:(e + 1) * FL],
                            start=(e == 0), stop=(e == NE - 1))
                    nc.vector.tensor_add(
                        vt[j].rearrange("p (h w) -> p h w", w=65)[:, :, 0:64],
                        ps.rearrange("p (h w) -> p h w", w=64),
                        bv_sb.rearrange("p (h w) -> p h w", w=64))
                tc.cur_priority = save

            # ---- projections (all emitted up front, need-ordered).
            # K/Q f0 of chunk 0 first so the first S-pair unblocks after
            # ~16 matmuls instead of 64.
            nc.sync.dma_start(
                out=wk_sb.rearrange("p (e f) -> p e f", f=FL),
                in_=wkT.rearrange("(e p) f -> p e f", p=128))
            xk0 = load_chunk(xkp, kT3, 0)
            nc.sync.dma_start(
                out=wq_sb.rearrange("p (e f) -> p e f", f=FL),
                in_=wqT.rearrange("(e p) f -> p e f", p=128))
            xq0 = load_chunk(xqp, qT3, 0)
            qk_proj(0, xk0, wk_sb, bk_sb, kt, [0], PRI_PROJ)
            qk_proj(0, xq0, wq_sb, bq_sb, qt, [0], PRI_PROJ)
            qk_proj(0, xk0, wk_sb, bk_sb, kt, [1, 2, 3], PRI_PROJ)
            qk_proj(0, xq0, wq_sb, bq_sb, qt, [1, 2, 3], PRI_PROJ)
            nc.sync.dma_start(
                out=wv_sb.rearrange("p (e f) -> p e f", f=FL),
                in_=wvT.rearrange("(e p) f -> p e f", p=128))
            xk1 = load_chunk(xkp, kT3, 1)
            qk_proj(1, xk1, wk_sb, bk_sb, kt, [0, 1, 2, 3], PRI_PROJ)
            xv0 = load_chunk(xvp, vT3, 0)
            v_proj(0, xv0, PRI_PROJ)
            xk2 = load_chunk(xkp, kT3, 2)
            qk_proj(2, xk2, wk_sb, bk_sb, kt, [0, 1, 2, 3], PRI_PROJ)
            xv1 = load_chunk(xvp, vT3, 1)
            v_proj(1, xv1, PRI_PROJ)
            xk3 = load_chunk(xkp, kT3, 3)
            qk_proj(3, xk3, wk_sb, bk_sb, kt, [0, 1, 2, 3], PRI_PROJ)
            xv2 = load_chunk(xvp, vT3, 2)
            v_proj(2, xv2, PRI_PROJ)
            xv3 = load_chunk(xvp, vT3, 3)
            v_proj(3, xv3, PRI_PROJ)
            nc.sync.dma_start(
                out=wo_sb.rearrange("p (f e) -> p f e", e=E),
                in_=woT.rearrange("(f p) e -> p f e", p=128))
            for c in range(1, NC):
                xq = load_chunk(xqp, qT3, c)
                qk_proj(c, xq, wq_sb, bq_sb, qt, [0, 1, 2, 3], PRI_PROJ)

            # ---- attention + output projection ----
            def unit(hp, qc):
                """One (head-pair, q-chunk-512) attention unit."""
                qsl = slice(qc * 512, (qc + 1) * 512)
                psoA = ps_o.tile([65, 512], F32, tag="psoA", name="psoA")
                psoB = ps_o.tile([65, 512], F32, tag="psoB", name="psoB")
                for k in range(NT):
                    pss = ps_s.tile([128, 1024], F32, tag="pss",
                                    name="pss")
                    for t in range(2):
                        r = slice(t * 64, (t + 1) * 64)
                        nc.tensor.matmul(
                            pss[:, t * 512:(t + 1) * 512],
                            lhsT=kt[hp][r, k * 128:(k + 1) * 128],
                            rhs=qt[hp][r, qsl],
                            start=True, stop=True,
                            tile_position=(t * 64, 0))
                    es = esp.tile([128, 1024], DT, tag="es", name="es")
                    nc.scalar.activation(
                        out=es[:], in_=pss[:],
                        func=mybir.ActivationFunctionType.Exp,
                        bias=mb_sb[:, k:k + 1], scale=0.125)
                    h0 = 2 * hp
                    nc.tensor.matmul(
                        psoA[:], lhsT=vt[k][:, h0 * 65:(h0 + 1) * 65],
                        rhs=es[:, 0:512],
                        start=(k == 0), stop=(k == NT - 1))
                    nc.tensor.matmul(
                        psoB[:], lhsT=vt[k][:, (h0 + 1) * 65:(h0 + 2) * 65],
                        rhs=es[:, 512:1024],
                        start=(k == 0), stop=(k == NT - 1))
                # Copy the accumulators to SBUF right away so the single
                # pso PSUM buffer is freed for the next unit; the whole
                # normalization chain then runs from SBUF off the PSUM
                # critical path. Rows 0..63 = O^T, row 64 = sum(exp).
                ot = normp.tile([65, 1024], F32, tag="ot", name="ot")
                nc.vector.tensor_copy(out=ot[:, 0:512], in_=psoA[:])
                nc.vector.tensor_copy(out=ot[:, 512:1024], in_=psoB[:])
                # The 4-hop DRAM bounce (repartition sums row -> 64-lane
                # reciprocal -> partition broadcast) runs on the
                # otherwise-idle GpSimd SWDGE queue and at demoted
                # priority: it gates only the output projection, and it
                # must never block the Sync DMA queue or the DVE ahead
                # of the next unit's pso-freeing copies.
                save = demote(PRI_NORM)
                rsd = normdp.tile([1, 1024], F32, tag="rsd", name="rsd")
                nc.gpsimd.dma_start(out=rsd[:], in_=ot[64:65, :])
                rs = normp.tile([64, 16], F32, tag="rs", name="rs")
                nc.gpsimd.dma_start(
                    out=rs[:],
                    in_=rsd.rearrange("o (p w) -> (o p) w", w=16))
                ri = normp.tile([64, 16], F32, tag="ri", name="ri")
                nc.vector.reciprocal(ri[:], rs[:])
                rid = normdp.tile([64, 16], F32, tag="rid", name="rid")
                nc.gpsimd.dma_start(out=rid[:], in_=ri[:])
                rep = normp.tile([64, 1024], F32, tag="rep", name="rep")
                nc.gpsimd.dma_start(
                    out=rep[:],
                    in_=rid.rearrange("p w -> () (p w)").to_broadcast(
                        [64, 1024]))
                nc.vector.tensor_mul(
                    xtl[hp][0:64, qsl], ot[0:64, 0:512], rep[:, 0:512])
                nc.vector.tensor_mul(
                    xtl[hp][64:128, qsl], ot[0:64, 512:1024],
                    rep[:, 512:1024])
                tc.cur_priority = save

            def final_group(j, c2):
                """Output projection for q rows j*128.. and E half c2."""
                save = demote(PRI_FINAL)
                ps = ps_w.tile([128, 512], F32, tag="psw", name="psf")
                for f in range(HP):
                    nc.tensor.matmul(
                        ps[:],
                        lhsT=xtl[f][:, j * 128:(j + 1) * 128],
                        rhs=wo_sb[:, f * E + c2 * 512:
                                  f * E + (c2 + 1) * 512],
                        start=(f == 0), stop=(f == HP - 1))
                ob = obp.tile([128, 512], F32, tag="ob", name="ob")
                nc.vector.tensor_add(
                    ob[:], ps[:], bo_sb[:, c2 * 512:(c2 + 1) * 512])
                nc.sync.dma_start(
                    out=out[j * 128:(j + 1) * 128,
                            c2 * 512:(c2 + 1) * 512],
                    in_=ob[:])
                tc.cur_priority = save

            # Emit each qc's output projection interleaved into the NEXT
            # qc's units so its (demoted) instructions never sit at a
            # unit boundary waiting on the normalization chain.
            pending = []
            for qc in range(NC):
                for hp in range(HP):
                    unit(hp, qc)
                    for _ in range(2):
                        if pending:
                            final_group(*pending.pop(0))
                pending = [(j, c2) for j in range(4 * qc, 4 * qc + 4)
                           for c2 in range(2)]
            for grp in pending:
                final_group(*grp)

    nc.compile()
    return nc


_NC_CACHE = None


def _get_nc():
    global _NC_CACHE
    if _NC_CACHE is None:
        _NC_CACHE = build_nc()
    return _NC_CACHE


def make_in_maps(query, key_, value, mask, w_q, b_q, w_k, b_k, w_v, b_v,
                 w_o, b_o):
    import ml_dtypes
    f32 = np.float32
    bf16 = ml_dtypes.bfloat16
    c = lambda a: np.ascontiguousarray(a).astype(bf16)
    in_maps = []
    for core in range(N_CORES):
        b, g = core // 2, core % 2
        fs = slice(g * FL, (g + 1) * FL)
        mb = np.where(np.asarray(mask[b]).reshape(NT, 128).T,
                      0.0, -30.0).astype(f32)
        in_maps.append({
            "qT": c(query[b].T.astype(f32, copy=False)),
            "kT": c(key_[b].T.astype(f32, copy=False)),
            "vT": c(value[b].T.astype(f32, copy=False)),
            "wqT": c(w_q[fs, :].T.astype(f32, copy=False)),
            "wkT": c(w_k[fs, :].T.astype(f32, copy=False)),
            "wvT": c(w_v[fs, :].T.astype(f32, copy=False)),
            "woT": c(w_o[:, fs].T.astype(f32, copy=False)),
            "bqc": np.ascontiguousarray(
                b_q[fs].astype(f32, copy=False).reshape(HP, 128).T),
            "bkc": np.ascontiguousarray(
                b_k[fs].astype(f32, copy=False).reshape(HP, 128).T),
            "bvr": c(np.broadcast_to(b_v[fs], (128, FL))),
            "bor": c(np.broadcast_to(
                b_o if g == 0 else np.zeros(E, f32), (128, E))),
            "mbd": np.ascontiguousarray(mb),
        })
    return in_maps


def kernel(query=None, key_=None, value=None, mask=None, w_q=None, b_q=None,
           w_k=None, b_k=None, w_v=None, b_v=None, w_o=None, b_o=None,
           key=None, **_kwargs):
    if key_ is None:
        key_ = key
    args = [np.asarray(a) for a in
            (query, key_, value, mask, w_q, b_q, w_k, b_k, w_v, b_v,
             w_o, b_o)]
    nc = _get_nc()
    in_maps = make_in_maps(*args)
    res = run_bass_kernel_spmd(nc, in_maps, core_ids=list(range(N_CORES)))
    outs = [res.results[i]["out"] for i in range(N_CORES)]
    full = np.empty((B, T, E), np.float32)
    for b in range(B):
        full[b] = outs[2 * b] + outs[2 * b + 1]
    return full
